# revision 33
# baseline (speedup 1.0000x reference)
"""Trainium2 Bass kernel for BatchGroupItN (iterative whitening group norm).

Math (reference):
    x: (N=64, C=256, H=56, W=56) fp32.  Group of channel c is g = c % 32.
    xg[g, m] collects all elements with c % 32 == g  (m = 512*3136 per group).
    sigma = cov(xg) + eps*I  (32x32); wm = sigma^{-1/2} via 5 Newton-Schulz
    iters on trace-normalized sigma; out = (wm @ (xg - mu)) scattered back,
    then * weight + bias.

Strategy (8 cores, data-parallel over batch N):
    Each core owns 8 batches = 16 contiguous slabs of [128 channels, 3136 hw].
    Channel partition p of a slab belongs to group p % 32.
    Pass 1 (bf16): cast each slab to bf16, PE-transpose [128,128] chunks ->
    T [m,c] tiles, Gram matmuls accumulate S128 = sum T^T T in PSUM; the four
    32x32 diagonal blocks of S128 sum to S = sum x x^T.  Channel sums come
    from an in-place fp32 ACT copy with accum_out (full precision).
    Fold S128/sums to 32-wide via selector matmuls, AllReduce a packed
    [32,64] buffer across the 8 cores, then every core runs the (tiny)
    Newton-Schulz iterations and builds a block-diagonal WM = diag(wm x4).
    Pass 2 (fp32): y = WM @ x per [128,512] chunk on the PE, then one
    per-partition affine (scale=weight, bias=bias - wm@mu * weight,
    alternating DVE/ACT) and DMA out.  The first RESIDENT slabs stay in
    SBUF between passes; the rest are re-read from HBM.
"""

import numpy as np

import concourse.bass as bass
import concourse.bacc as bacc
import concourse.tile as tile
from concourse import bass_utils, mybir

F32 = mybir.dt.float32
BF16 = mybir.dt.bfloat16
AX = mybir.AxisListType
OP = mybir.AluOpType
AF = mybir.ActivationFunctionType

N_CORES = 8
G = 32
T_ITERS = 5
EPS = 1e-5
N, C, H, W = 64, 256, 56, 56
HW = H * W  # 3136
P = 128
SLABS = 16  # per core: 8 batches x 2 channel-halves of 128
M_TOTAL = float(N * (C // G) * HW)  # 1,605,632 elements per group
RESIDENT = 10  # slabs kept in SBUF between pass 1 and pass 2
GRPS = (HW + 511) // 512  # 7: six full 512 groups + one 64 tail
N_WARM = 24  # dummy matmuls keeping the PE warm through the all-reduce


def _emit(ctx, tc, x, w2, b2, i128, bd, bdm, out):
    nc = tc.nc

    consts = ctx.enter_context(tc.tile_pool(name="consts", bufs=1))
    single = ctx.enter_context(tc.tile_pool(name="single", bufs=1))
    ns = ctx.enter_context(tc.tile_pool(name="ns", bufs=3))
    xres = ctx.enter_context(tc.tile_pool(name="xres", bufs=RESIDENT))
    xstream = ctx.enter_context(tc.tile_pool(name="xstream", bufs=2))
    xbp = ctx.enter_context(tc.tile_pool(name="xbp", bufs=2))
    tp = ctx.enter_context(tc.tile_pool(name="tp", bufs=3))
    outp = ctx.enter_context(tc.tile_pool(name="outp", bufs=4))
    psA = ctx.enter_context(tc.tile_pool(name="psA", bufs=1, space="PSUM"))
    # one shared 6-deep psum pool: pass-1 transpose staging and pass-2 matmul
    # outputs use the same slots (tag "ps"), so both passes pipeline 6 deep
    psB = ctx.enter_context(tc.tile_pool(name="psB", bufs=5, space="PSUM"))
    psS = ctx.enter_context(tc.tile_pool(name="psS", bufs=1, space="PSUM"))
    psS2 = ctx.enter_context(tc.tile_pool(name="psS2", bufs=1, space="PSUM"))
    dram = ctx.enter_context(tc.tile_pool(name="dram", bufs=1, space="DRAM"))

    I128 = consts.tile([P, P], F32)
    nc.sync.dma_start(I128, i128)
    I128b = consts.tile([P, P], BF16)
    nc.vector.tensor_copy(I128b, I128)
    BD = consts.tile([P, P], F32)
    nc.sync.dma_start(BD, bd)
    BDM = consts.tile([P, P], F32)
    nc.sync.dma_start(BDM, bdm)
    I32 = I128[0:G, 0:G]
    ones = consts.tile([P, G], F32)
    nc.vector.memset(ones, 1.0)
    wsb = consts.tile([P, 2], F32)
    bsb = consts.tile([P, 2], F32)
    for h in range(2):
        nc.sync.dma_start(wsb[:, h : h + 1], w2[h])
        nc.sync.dma_start(bsb[:, h : h + 1], b2[h])
    negepsI = consts.tile([G, G], F32)
    nc.vector.tensor_scalar_mul(negepsI, I32, -EPS)
    pack = single.tile([G, 64], F32)
    nc.vector.memset(pack, 0.0)

    # ---------------- pass 1: statistics (bf16 compute) ---------
    # psum_S cols 0:128 accumulate S128 = sum T^T T; col 128 accumulates the
    # channel sums (each Gram's rhs is [T_chunk | ones], one extra column).
    psum_S = psA.tile([P, 136], F32, tag="pS")

    xt_tiles = [None] * SLABS
    n_grams = SLABS * 25  # 6 groups x 4 chunks + 1 tail chunk, per slab
    gram_i = 0
    copy_i = 0
    for s in range(SLABS):
        if s < RESIDENT:
            xt = xres.tile([P, HW], F32, tag="xr")
        else:
            xt = xstream.tile([P, HW], F32, tag="xs")
        xt_tiles[s] = xt
        nc.sync.dma_start(xt, x[s])
        xb = xbp.tile([P, HW], BF16, tag="xb")
        nc.vector.tensor_copy(xb, xt)  # fp32 -> bf16 cast
        for grp in range(GRPS):
            off = 512 * grp
            wd = min(512, HW - off)  # 512 or 64
            nch = (wd + 127) // 128  # 4 or 1
            pt = psB.tile([P, 512], BF16, tag="ps")
            for k in range(nch):
                cw = min(128, wd - 128 * k)  # 128 or 64
                nc.tensor.transpose(
                    pt[0:cw, 128 * k : 128 * k + P],
                    xb[:, off + 128 * k : off + 128 * k + cw],
                    I128b,
                )
            # tsb chunk k occupies [:, k, 0:128]; [:, k, 128] is a ones
            # column so the Gram rhs [T_k | 1] yields channel sums for free
            tsb = tp.tile([P, 4, 132], BF16, tag="tsb")
            copy_i += 1
            eng = nc.vector if copy_i % 2 == 0 else nc.scalar
            if wd == 512:
                if eng is nc.vector:
                    nc.vector.tensor_copy(tsb[:, :, 0:P], pt)
                else:
                    nc.scalar.copy(tsb[:, :, 0:P], pt)
                nc.vector.memset(tsb[:, :, P : P + 1], 1.0)
            else:
                if eng is nc.vector:
                    nc.vector.tensor_copy(tsb[0:wd, 0, 0:P], pt[0:wd, 0:P])
                else:
                    nc.scalar.copy(tsb[0:wd, 0, 0:P], pt[0:wd, 0:P])
                nc.vector.memset(tsb[0:wd, 0, P : P + 1], 1.0)
            for k in range(nch):
                cw = min(128, wd - 128 * k)
                gram_i += 1
                nc.tensor.matmul(
                    psum_S[:, 0 : P + 1],
                    lhsT=tsb[0:cw, k, 0:P],
                    rhs=tsb[0:cw, k, 0 : P + 1],
                    start=(gram_i == 1),
                    stop=(gram_i == n_grams),
                )

    # ---------------- fold + all-reduce ----------------
    Ssb = single.tile([P, 136], F32)
    nc.vector.tensor_copy(Ssb[:, 0 : P + 1], psum_S[:, 0 : P + 1])
    sums128 = Ssb[:, P : P + 1]
    ps32 = psS.tile([G, 64], F32, tag="sps")
    for i in range(4):
        # lhsT = columns of I128: selects ONLY row-block i of S128
        nc.tensor.matmul(
            ps32[:, 0:G],
            lhsT=I128[:, G * i : G * i + G],
            rhs=Ssb[:, G * i : G * i + G],
            start=(i == 0),
            stop=(i == 3),
        )
    nc.tensor.matmul(ps32[:, G : G + 1], lhsT=BD[:, 0:G], rhs=sums128, start=True, stop=True)
    nc.vector.tensor_copy(pack[:, 0 : G + 1], ps32[:, 0 : G + 1])

    cc_in = dram.tile([G, 64], F32)
    cc_out = dram.tile([G, 64], F32)
    nc.sync.dma_start(cc_in, pack)
    nc.gpsimd.collective_compute(
        "AllReduce",
        OP.add,
        replica_groups=[list(range(N_CORES))],
        ins=[cc_in.opt()],
        outs=[cc_out.opt()],
    )

    # keep the PE's HAM clock warm through the all-reduce wait: identity
    # matmuls on already-resident data into otherwise-idle psum banks
    warm_src = xt_tiles[0]
    for wi in range(N_WARM):
        pw = psB.tile([P, 512], F32, tag="ps")
        nc.tensor.matmul(
            pw, lhsT=I128, rhs=warm_src[:, 0:512], start=True, stop=True
        )

    packr = single.tile([G, 64], F32)
    nc.sync.dma_start(packr, cc_out)

    # post-all-reduce warm-up: depends on packr, so these run right after the
    # collective lands, re-warming the PE while the stats chain executes
    for wi in range(16):
        pw = psB.tile([P, 512], F32, tag="ps")
        nc.tensor.matmul(
            pw[0:64, :], lhsT=packr, rhs=warm_src[0:G, 0:512], start=True, stop=True
        )

    # ---------------- sigma, trace, Newton-Schulz ----------------
    # Rescaled NS iteration: with P_k = 1.5^k Q_k,
    #   Q_{k+1} = Q_k - Q_k^3 (0.5 * 1.5^(2k-1) * sigma_N),  Q_0 = I
    # and wm = 1.5^5 Q_5 sqrt(tinv), folded as sqrt(1.5^10 * tinv).
    inv_m = 1.0 / M_TOTAL
    mu = single.tile([G, 1], F32)
    nc.vector.tensor_scalar_mul(mu, packr[:, G : G + 1], inv_m)
    ps_mr = psS.tile([1, G], F32, tag="sps")
    nc.tensor.transpose(ps_mr, mu, I32)
    murow = single.tile([1, G], F32)
    nc.vector.tensor_copy(murow, ps_mr)
    # ps_mm = mu mu^T - eps*I  (second accumulating matmul adds -eps*I)
    ps_mm = psS.tile([G, G], F32, tag="sps")
    nc.tensor.matmul(ps_mm, lhsT=murow, rhs=murow, start=True, stop=False)
    nc.tensor.matmul(ps_mm, lhsT=negepsI, rhs=I32, start=False, stop=True)
    sigma = single.tile([G, G], F32)
    nc.vector.tensor_scalar_mul(sigma, packr[:, 0:G], inv_m)
    nc.vector.tensor_sub(sigma, sigma, ps_mm)

    diag = single.tile([G, G], F32)
    nc.vector.tensor_mul(diag, sigma, I32)
    dcol = single.tile([G, 1], F32)
    nc.vector.tensor_reduce(dcol, diag, AX.X, OP.add)
    ps_tr = psS.tile([1, 1], F32, tag="sps")
    nc.tensor.matmul(ps_tr, lhsT=dcol, rhs=ones[0:G, 0:1], start=True, stop=True)
    trsb = single.tile([1, 1], F32)
    nc.vector.tensor_copy(trsb, ps_tr)
    tinv = single.tile([1, 1], F32)
    nc.vector.reciprocal(tinv, trsb)
    ps_b32 = psS.tile([G, 1], F32, tag="sps")
    nc.tensor.matmul(ps_b32, lhsT=ones[0:1, 0:G], rhs=tinv, start=True, stop=True)
    tinv32 = single.tile([G, 1], F32)
    nc.vector.tensor_copy(tinv32, ps_b32)
    # stinv = sqrt(1.5^10 * tinv)  (per-partition broadcast)
    tinv57 = single.tile([G, 1], F32)
    nc.vector.tensor_scalar_mul(tinv57, tinv32, 1.5**10)
    stinv = single.tile([G, 1], F32)
    nc.scalar.sqrt(stinv, tinv57)
    # pre-scaled sigma_k = sigma * tinv * (0.5 * 1.5^(2k-1))
    sigks = []
    for k in range(T_ITERS):
        sk = single.tile([G, G], F32, tag=f"sigk{k}")
        nc.vector.tensor_scalar(
            out=sk, in0=sigma, scalar1=tinv32, scalar2=0.5 * 1.5 ** (2 * k - 1),
            op0=OP.mult, op1=OP.mult,
        )
        sigks.append(sk)

    # NS loop with 4 cross-engine transitions per iteration:
    #   PE: A_ps = Q^2, B_ps = Q sig_k (independent, back-to-back)
    #   DVE: copy A, B to SBUF;  PE: C_ps = A^T B = Q^3 sig_k;  DVE: Q -= C
    Pcur = single.tile([G, G], F32, tag="P0")
    nc.vector.tensor_copy(Pcur, I32)
    wi = 0
    for k in range(T_ITERS):
        psa = psS.tile([G, G], F32, tag="sps")
        nc.tensor.matmul(psa, lhsT=Pcur, rhs=Pcur, start=True, stop=True)
        psb_ = psS2.tile([G, G], F32, tag="sps2")
        nc.tensor.matmul(psb_, lhsT=Pcur, rhs=sigks[k], start=True, stop=True)
        asb = ns.tile([G, G], F32, tag="nsA")
        nc.vector.tensor_copy(asb, psa)
        bsb_ = ns.tile([G, G], F32, tag="nsB")
        nc.vector.tensor_copy(bsb_, psb_)
        # keep the PE's HAM clock warm while DVE runs the copies
        for _ in range(2):
            pw = psB.tile([P, 512], F32, tag="ps")
            nc.tensor.matmul(
                pw[0:64, :], lhsT=packr, rhs=warm_src[0:G, 0:512],
                start=True, stop=True,
            )
        psc = psS.tile([G, G], F32, tag="sps")
        nc.tensor.matmul(psc, lhsT=asb, rhs=bsb_, start=True, stop=True)
        pn = ns.tile([G, G], F32, tag="nsP")
        nc.vector.tensor_sub(pn, Pcur, psc)
        Pcur = pn

    wm = single.tile([G, G], F32)
    nc.vector.tensor_scalar_mul(wm, Pcur, stinv)

    # block-diagonal WM = diag(wm, wm, wm, wm): write the four diagonal
    # blocks on the PE, then one masked copy (off-diag psum garbage is
    # finite leftovers from the warm-up matmuls, zeroed by the mask)
    ps_wm = psB.tile([P, P], F32, tag="ps")
    for i in range(4):
        nc.tensor.matmul(
            ps_wm[G * i : G * i + G, G * i : G * i + G],
            lhsT=wm,
            rhs=I32,
            start=True,
            stop=True,
            tile_position=(0, G * i),
        )
    WM = single.tile([P, P], F32)
    nc.vector.tensor_mul(WM, ps_wm, BDM)

    # per-partition affine: scale = weight, bias = bias - (wm @ mu) * weight
    ps_v = psS.tile([G, 1], F32, tag="sps")
    nc.tensor.matmul(ps_v, lhsT=wm, rhs=mu, start=True, stop=True)
    vsb = single.tile([G, 1], F32)
    nc.vector.tensor_copy(vsb, ps_v)
    ps_v128 = psS.tile([P, 1], F32, tag="sps")
    nc.tensor.matmul(ps_v128, lhsT=BD[0:G, :], rhs=vsb, start=True, stop=True)
    v128 = single.tile([P, 1], F32)
    nc.vector.tensor_copy(v128, ps_v128)
    badj = single.tile([P, 2], F32)
    for h in range(2):
        nc.vector.tensor_mul(badj[:, h : h + 1], v128, wsb[:, h : h + 1])
        nc.vector.tensor_sub(badj[:, h : h + 1], bsb[:, h : h + 1], badj[:, h : h + 1])

    # ---------------- pass 2: normalize (fp32) ----------------
    # order: last two streamed slabs first (still resident in xstream slots);
    # re-read slabs spaced every third so write+read HBM demand stays under
    # the per-core bandwidth
    order = [14, 15, 0, 10, 1, 2, 11, 3, 4, 12, 5, 6, 13, 7, 8, 9]
    HALF_A = 1536  # chunks 0..2; chunks 3..6 cover 1536:3136 (1600 cols)
    for s in order:
        if RESIDENT <= s < SLABS - 2:
            xt = xstream.tile([P, HW], F32, tag="xs")
            nc.sync.dma_start(xt, x[s])
        else:
            xt = xt_tiles[s]
        h = s % 2
        osb_a = outp.tile([P, HW - HALF_A], F32, tag="osb")
        osb_b = outp.tile([P, HW - HALF_A], F32, tag="osb")
        for grp in range(GRPS):
            off = 512 * grp
            wd = min(512, HW - off)
            osb, ooff = (osb_a, off) if off < HALF_A else (osb_b, off - HALF_A)
            py = psB.tile([P, 512], F32, tag="ps")
            nc.tensor.matmul(
                py[:, 0:wd], lhsT=WM, rhs=xt[:, off : off + wd], start=True, stop=True
            )
            if grp % 2 == 0:
                nc.scalar.activation(
                    out=osb[:, ooff : ooff + wd],
                    in_=py[:, 0:wd],
                    func=AF.Identity,
                    bias=badj[:, h : h + 1],
                    scale=wsb[:, h : h + 1],
                )
            else:
                nc.vector.tensor_scalar(
                    out=osb[:, ooff : ooff + wd],
                    in0=py[:, 0:wd],
                    scalar1=wsb[:, h : h + 1],
                    scalar2=badj[:, h : h + 1],
                    op0=OP.mult,
                    op1=OP.add,
                )
        nc.sync.dma_start(out[s, :, 0:HALF_A], osb_a[:, 0:HALF_A])
        nc.sync.dma_start(out[s, :, HALF_A:HW], osb_b[:, 0 : HW - HALF_A])


_BUILT = None


def _build():
    global _BUILT
    if _BUILT is not None:
        return _BUILT
    nc = bacc.Bacc(
        "TRN2",
        target_bir_lowering=False,
        debug=False,
        enable_asserts=False,
        num_devices=N_CORES,
    )
    x_d = nc.dram_tensor("x", [SLABS, P, HW], F32, kind="ExternalInput")
    w_d = nc.dram_tensor("w2", [2, P, 1], F32, kind="ExternalInput")
    b_d = nc.dram_tensor("b2", [2, P, 1], F32, kind="ExternalInput")
    i_d = nc.dram_tensor("i128", [P, P], F32, kind="ExternalInput")
    bd_d = nc.dram_tensor("bd128", [P, P], F32, kind="ExternalInput")
    bdm_d = nc.dram_tensor("bdm128", [P, P], F32, kind="ExternalInput")
    o_d = nc.dram_tensor("out", [SLABS, P, HW], F32, kind="ExternalOutput")
    from contextlib import ExitStack

    with tile.TileContext(nc) as tc, ExitStack() as ctx:
        _emit(
            ctx, tc, x_d.ap(), w_d.ap(), b_d.ap(), i_d.ap(), bd_d.ap(),
            bdm_d.ap(), o_d.ap(),
        )
    nc.compile()
    _BUILT = nc
    return nc


def kernel(x, weight, bias, trace=False, tmpdir=None):
    x = np.ascontiguousarray(np.asarray(x, dtype=np.float32))
    weight = np.asarray(weight, dtype=np.float32)
    bias = np.asarray(bias, dtype=np.float32)
    assert x.shape == (N, C, H, W)

    nc = _build()

    w2 = np.ascontiguousarray(weight.reshape(2, P, 1))
    b2 = np.ascontiguousarray(bias.reshape(2, P, 1))
    i128 = np.eye(P, dtype=np.float32)
    idx = np.arange(P)
    bd128 = (idx[:, None] % G == idx[None, :] % G).astype(np.float32)
    bdm128 = (idx[:, None] // G == idx[None, :] // G).astype(np.float32)

    xs = x.reshape(N_CORES, SLABS, P, HW)
    in_maps = [
        {
            "x": xs[c], "w2": w2, "b2": b2, "i128": i128, "bd128": bd128,
            "bdm128": bdm128,
        }
        for c in range(N_CORES)
    ]
    res = bass_utils.run_bass_kernel_spmd(
        nc, in_maps, core_ids=list(range(N_CORES)), trace=trace, tmpdir=tmpdir
    )
    out = np.concatenate(
        [r["out"].reshape(1, N // N_CORES, C, H, W) for r in res.results], axis=0
    ).reshape(N, C, H, W)
    if trace:
        return out, res
    return out


# revision 38
# speedup vs baseline: 1.0914x; 1.0914x over previous
"""Trainium2 Bass kernel for BatchGroupItN (iterative whitening group norm).

Math (reference):
    x: (N=64, C=256, H=56, W=56) fp32.  Group of channel c is g = c % 32.
    xg[g, m] collects all elements with c % 32 == g  (m = 512*3136 per group).
    sigma = cov(xg) + eps*I  (32x32); wm = sigma^{-1/2} via 5 Newton-Schulz
    iters on trace-normalized sigma; out = (wm @ (xg - mu)) scattered back,
    then * weight + bias.

Strategy (8 cores, data-parallel over batch N):
    Each core owns 8 batches = 16 contiguous slabs of [128 channels, 3136 hw].
    Channel partition p of a slab belongs to group p % 32.
    Pass 1 (bf16): cast each slab to bf16, PE-transpose [128,128] chunks ->
    T [m,c] tiles, Gram matmuls accumulate S128 = sum T^T T in PSUM; the four
    32x32 diagonal blocks of S128 sum to S = sum x x^T.  Channel sums come
    from an in-place fp32 ACT copy with accum_out (full precision).
    Fold S128/sums to 32-wide via selector matmuls, AllReduce a packed
    [32,64] buffer across the 8 cores, then every core runs the (tiny)
    Newton-Schulz iterations and builds a block-diagonal WM = diag(wm x4).
    Pass 2 (fp32): y = WM @ x per [128,512] chunk on the PE, then one
    per-partition affine (scale=weight, bias=bias - wm@mu * weight,
    alternating DVE/ACT) and DMA out.  The first RESIDENT slabs stay in
    SBUF between passes; the rest are re-read from HBM.
"""

import numpy as np

import concourse.bass as bass
import concourse.bacc as bacc
import concourse.tile as tile
from concourse import bass_utils, mybir

F32 = mybir.dt.float32
BF16 = mybir.dt.bfloat16
AX = mybir.AxisListType
OP = mybir.AluOpType
AF = mybir.ActivationFunctionType

N_CORES = 8
G = 32
T_ITERS = 5
EPS = 1e-5
N, C, H, W = 64, 256, 56, 56
HW = H * W  # 3136
P = 128
SLABS = 16  # per core: 8 batches x 2 channel-halves of 128
M_TOTAL = float(N * (C // G) * HW)  # 1,605,632 elements per group
RESIDENT = 10  # slabs kept in SBUF between pass 1 and pass 2
GRPS = (HW + 511) // 512  # 7: six full 512 groups + one 64 tail
N_WARM = 24  # dummy matmuls keeping the PE warm through the all-reduce


def _emit(ctx, tc, x, w2, b2, i128, bd, bdm, out):
    nc = tc.nc

    consts = ctx.enter_context(tc.tile_pool(name="consts", bufs=1))
    single = ctx.enter_context(tc.tile_pool(name="single", bufs=1))
    ns = ctx.enter_context(tc.tile_pool(name="ns", bufs=3))
    xres = ctx.enter_context(tc.tile_pool(name="xres", bufs=RESIDENT))
    xstream = ctx.enter_context(tc.tile_pool(name="xstream", bufs=2))
    xbp = ctx.enter_context(tc.tile_pool(name="xbp", bufs=2))
    tp = ctx.enter_context(tc.tile_pool(name="tp", bufs=3))
    outp = ctx.enter_context(tc.tile_pool(name="outp", bufs=4))
    psA = ctx.enter_context(tc.tile_pool(name="psA", bufs=1, space="PSUM"))
    # one shared 6-deep psum pool: pass-1 transpose staging and pass-2 matmul
    # outputs use the same slots (tag "ps"), so both passes pipeline 6 deep
    psB = ctx.enter_context(tc.tile_pool(name="psB", bufs=6, space="PSUM"))
    psS = ctx.enter_context(tc.tile_pool(name="psS", bufs=1, space="PSUM"))
    dram = ctx.enter_context(tc.tile_pool(name="dram", bufs=1, space="DRAM"))

    I128 = consts.tile([P, P], F32)
    nc.sync.dma_start(I128, i128)
    I128b = consts.tile([P, P], BF16)
    nc.vector.tensor_copy(I128b, I128)
    BD = consts.tile([P, P], F32)
    nc.sync.dma_start(BD, bd)
    BDM = consts.tile([P, P], F32)
    nc.sync.dma_start(BDM, bdm)
    I32 = I128[0:G, 0:G]
    ones = consts.tile([P, G], F32)
    nc.vector.memset(ones, 1.0)
    wsb = consts.tile([P, 2], F32)
    bsb = consts.tile([P, 2], F32)
    for h in range(2):
        nc.sync.dma_start(wsb[:, h : h + 1], w2[h])
        nc.sync.dma_start(bsb[:, h : h + 1], b2[h])
    negepsI = consts.tile([G, G], F32)
    nc.vector.tensor_scalar_mul(negepsI, I32, -EPS)
    pack = single.tile([G, 64], F32)
    nc.vector.memset(pack, 0.0)

    # ---------------- pass 1: statistics (bf16 compute) ---------
    # psum_S cols 0:128 accumulate S128 = sum T^T T; col 128 accumulates the
    # channel sums (each Gram's rhs is [T_chunk | ones], one extra column).
    psum_S = psA.tile([P, 136], F32, tag="pS")

    xt_tiles = [None] * SLABS
    n_grams = SLABS * 25  # 6 groups x 4 chunks + 1 tail chunk, per slab
    gram_i = 0
    copy_i = 0
    for s in range(SLABS):
        if s < RESIDENT:
            xt = xres.tile([P, HW], F32, tag="xr")
        else:
            xt = xstream.tile([P, HW], F32, tag="xs")
        xt_tiles[s] = xt
        nc.sync.dma_start(xt, x[s])
        xb = xbp.tile([P, HW], BF16, tag="xb")
        nc.vector.tensor_copy(xb, xt)  # fp32 -> bf16 cast
        for grp in range(GRPS):
            off = 512 * grp
            wd = min(512, HW - off)  # 512 or 64
            nch = (wd + 127) // 128  # 4 or 1
            pt = psB.tile([P, 512], BF16, tag="ps")
            for k in range(nch):
                cw = min(128, wd - 128 * k)  # 128 or 64
                nc.tensor.transpose(
                    pt[0:cw, 128 * k : 128 * k + P],
                    xb[:, off + 128 * k : off + 128 * k + cw],
                    I128b,
                )
            # tsb chunk k occupies [:, k, 0:128]; [:, k, 128] is a ones
            # column so the Gram rhs [T_k | 1] yields channel sums for free
            tsb = tp.tile([P, 4, 132], BF16, tag="tsb")
            copy_i += 1
            eng = nc.vector if copy_i % 2 == 0 else nc.scalar
            if wd == 512:
                if eng is nc.vector:
                    nc.vector.tensor_copy(tsb[:, :, 0:P], pt)
                else:
                    nc.scalar.copy(tsb[:, :, 0:P], pt)
                nc.vector.memset(tsb[:, :, P : P + 1], 1.0)
            else:
                if eng is nc.vector:
                    nc.vector.tensor_copy(tsb[0:wd, 0, 0:P], pt[0:wd, 0:P])
                else:
                    nc.scalar.copy(tsb[0:wd, 0, 0:P], pt[0:wd, 0:P])
                nc.vector.memset(tsb[0:wd, 0, P : P + 1], 1.0)
            for k in range(nch):
                cw = min(128, wd - 128 * k)
                gram_i += 1
                nc.tensor.matmul(
                    psum_S[:, 0 : P + 1],
                    lhsT=tsb[0:cw, k, 0:P],
                    rhs=tsb[0:cw, k, 0 : P + 1],
                    start=(gram_i == 1),
                    stop=(gram_i == n_grams),
                )

    # ---------------- fold + all-reduce ----------------
    Ssb = single.tile([P, 136], F32)
    nc.vector.tensor_copy(Ssb[:, 0 : P + 1], psum_S[:, 0 : P + 1])
    sums128 = Ssb[:, P : P + 1]
    ps32 = psS.tile([G, 64], F32, tag="sps")
    for i in range(4):
        # lhsT = columns of I128: selects ONLY row-block i of S128
        nc.tensor.matmul(
            ps32[:, 0:G],
            lhsT=I128[:, G * i : G * i + G],
            rhs=Ssb[:, G * i : G * i + G],
            start=(i == 0),
            stop=(i == 3),
        )
    nc.tensor.matmul(ps32[:, G : G + 1], lhsT=BD[:, 0:G], rhs=sums128, start=True, stop=True)
    nc.vector.tensor_copy(pack[:, 0 : G + 1], ps32[:, 0 : G + 1])

    cc_in = dram.tile([G, 64], F32)
    cc_out = dram.tile([G, 64], F32)
    nc.sync.dma_start(cc_in, pack)
    nc.gpsimd.collective_compute(
        "AllReduce",
        OP.add,
        replica_groups=[list(range(N_CORES))],
        ins=[cc_in.opt()],
        outs=[cc_out.opt()],
    )

    # keep the PE's HAM clock warm through the all-reduce wait: identity
    # matmuls on already-resident data into otherwise-idle psum banks
    warm_src = xt_tiles[0]
    for wi in range(8):
        pw = psB.tile([P, 512], F32, tag="ps")
        nc.tensor.matmul(
            pw, lhsT=I128, rhs=warm_src[:, 0:512], start=True, stop=True
        )

    packr = single.tile([G, 64], F32)
    nc.sync.dma_start(packr, cc_out)

    # post-all-reduce warm-up: depends on packr, so these run right after the
    # collective lands, re-warming the PE while the stats chain executes
    for wi in range(8):
        pw = psB.tile([P, 512], F32, tag="ps")
        nc.tensor.matmul(
            pw[0:64, :], lhsT=packr, rhs=warm_src[0:G, 0:512], start=True, stop=True
        )

    # ---------------- sigma, trace, Newton-Schulz ----------------
    # Rescaled NS iteration: with P_k = 1.5^k Q_k,
    #   Q_{k+1} = Q_k - Q_k^3 (0.5 * 1.5^(2k-1) * sigma_N),  Q_0 = I
    # and wm = 1.5^5 Q_5 sqrt(tinv), folded as sqrt(1.5^10 * tinv).
    inv_m = 1.0 / M_TOTAL
    mu = single.tile([G, 1], F32)
    nc.vector.tensor_scalar_mul(mu, packr[:, G : G + 1], inv_m)
    ps_mr = psS.tile([1, G], F32, tag="sps")
    nc.tensor.transpose(ps_mr, mu, I32)
    murow = single.tile([1, G], F32)
    nc.vector.tensor_copy(murow, ps_mr)
    # ps_mm = mu mu^T - eps*I  (second accumulating matmul adds -eps*I)
    ps_mm = psS.tile([G, G], F32, tag="sps")
    nc.tensor.matmul(ps_mm, lhsT=murow, rhs=murow, start=True, stop=False)
    nc.tensor.matmul(ps_mm, lhsT=negepsI, rhs=I32, start=False, stop=True)
    sigma = single.tile([G, G], F32)
    nc.vector.tensor_scalar_mul(sigma, packr[:, 0:G], inv_m)
    nc.vector.tensor_sub(sigma, sigma, ps_mm)

    diag = single.tile([G, G], F32)
    nc.vector.tensor_mul(diag, sigma, I32)
    dcol = single.tile([G, 1], F32)
    nc.vector.tensor_reduce(dcol, diag, AX.X, OP.add)
    ps_tr = psS.tile([1, 1], F32, tag="sps")
    nc.tensor.matmul(ps_tr, lhsT=dcol, rhs=ones[0:G, 0:1], start=True, stop=True)
    trsb = single.tile([1, 1], F32)
    nc.vector.tensor_copy(trsb, ps_tr)
    tinv = single.tile([1, 1], F32)
    nc.vector.reciprocal(tinv, trsb)
    ps_b32 = psS.tile([G, 1], F32, tag="sps")
    nc.tensor.matmul(ps_b32, lhsT=ones[0:1, 0:G], rhs=tinv, start=True, stop=True)
    tinv32 = single.tile([G, 1], F32)
    nc.vector.tensor_copy(tinv32, ps_b32)
    # stinv = sqrt(1.5^10 * tinv)  (per-partition broadcast)
    tinv57 = single.tile([G, 1], F32)
    nc.vector.tensor_scalar_mul(tinv57, tinv32, 1.5**10)
    stinv = single.tile([G, 1], F32)
    nc.scalar.sqrt(stinv, tinv57)
    # pre-scaled sigma_k = sigma * tinv * (0.5 * 1.5^(2k-1))
    sigks = []
    for k in range(T_ITERS):
        sk = single.tile([G, G], F32, tag=f"sigk{k}")
        nc.vector.tensor_scalar(
            out=sk, in0=sigma, scalar1=tinv32, scalar2=0.5 * 1.5 ** (2 * k - 1),
            op0=OP.mult, op1=OP.mult,
        )
        sigks.append(sk)

    # NS loop with 4 cross-engine transitions per iteration:
    #   PE: A_ps = Q^2, B_ps = Q sig_k (independent, back-to-back)
    #   DVE: copy A, B to SBUF;  PE: C_ps = A^T B = Q^3 sig_k;  DVE: Q -= C
    Pcur = single.tile([G, G], F32, tag="P0")
    nc.vector.tensor_copy(Pcur, I32)
    wi = 0
    for k in range(T_ITERS):
        psa = psS.tile([G, G], F32, tag="sps")
        nc.tensor.matmul(psa, lhsT=Pcur, rhs=Pcur, start=True, stop=True)
        psb_ = psB.tile([G, G], F32, tag="ps")
        nc.tensor.matmul(psb_, lhsT=Pcur, rhs=sigks[k], start=True, stop=True)
        asb = ns.tile([G, G], F32, tag="nsA")
        nc.vector.tensor_copy(asb, psa)
        bsb_ = ns.tile([G, G], F32, tag="nsB")
        nc.vector.tensor_copy(bsb_, psb_)
        # keep the PE's HAM clock warm while DVE runs the copies
        for _ in range(2):
            pw = psB.tile([P, 512], F32, tag="ps")
            nc.tensor.matmul(
                pw[0:64, :], lhsT=packr, rhs=warm_src[0:G, 0:512],
                start=True, stop=True,
            )
        psc = psS.tile([G, G], F32, tag="sps")
        nc.tensor.matmul(psc, lhsT=asb, rhs=bsb_, start=True, stop=True)
        pn = ns.tile([G, G], F32, tag="nsP")
        nc.vector.tensor_sub(pn, Pcur, psc)
        Pcur = pn

    wm = single.tile([G, G], F32)
    nc.vector.tensor_scalar_mul(wm, Pcur, stinv)

    # block-diagonal WM = diag(wm, wm, wm, wm): write the four diagonal
    # blocks on the PE, then one masked copy (off-diag psum garbage is
    # finite leftovers from the warm-up matmuls, zeroed by the mask)
    ps_wm = psB.tile([P, P], F32, tag="ps")
    for i in range(4):
        nc.tensor.matmul(
            ps_wm[G * i : G * i + G, G * i : G * i + G],
            lhsT=wm,
            rhs=I32,
            start=True,
            stop=True,
            tile_position=(0, G * i),
        )
    WM = single.tile([P, P], F32)
    nc.vector.tensor_mul(WM, ps_wm, BDM)

    # per-partition affine: scale = weight, bias = bias - (wm @ mu) * weight
    ps_v = psS.tile([G, 1], F32, tag="sps")
    nc.tensor.matmul(ps_v, lhsT=wm, rhs=mu, start=True, stop=True)
    vsb = single.tile([G, 1], F32)
    nc.vector.tensor_copy(vsb, ps_v)
    ps_v128 = psS.tile([P, 1], F32, tag="sps")
    nc.tensor.matmul(ps_v128, lhsT=BD[0:G, :], rhs=vsb, start=True, stop=True)
    v128 = single.tile([P, 1], F32)
    nc.vector.tensor_copy(v128, ps_v128)
    badj = single.tile([P, 2], F32)
    for h in range(2):
        nc.vector.tensor_mul(badj[:, h : h + 1], v128, wsb[:, h : h + 1])
        nc.vector.tensor_sub(badj[:, h : h + 1], bsb[:, h : h + 1], badj[:, h : h + 1])

    # ---------------- pass 2: normalize (fp32) ----------------
    # order: last two streamed slabs first (still resident in xstream slots);
    # re-read slabs spaced every third so write+read HBM demand stays under
    # the per-core bandwidth
    order = [14, 15, 0, 10, 1, 2, 11, 3, 4, 12, 5, 6, 13, 7, 8, 9]
    HALF_A = 1536  # chunks 0..2; chunks 3..6 cover 1536:3136 (1600 cols)
    for s in order:
        if RESIDENT <= s < SLABS - 2:
            xt = xstream.tile([P, HW], F32, tag="xs")
            nc.sync.dma_start(xt, x[s])
        else:
            xt = xt_tiles[s]
        h = s % 2
        osb_a = outp.tile([P, HW - HALF_A], F32, tag="osb")
        osb_b = outp.tile([P, HW - HALF_A], F32, tag="osb")
        for grp in range(GRPS):
            off = 512 * grp
            wd = min(512, HW - off)
            osb, ooff = (osb_a, off) if off < HALF_A else (osb_b, off - HALF_A)
            py = psB.tile([P, 512], F32, tag="ps")
            nc.tensor.matmul(
                py[:, 0:wd], lhsT=WM, rhs=xt[:, off : off + wd], start=True, stop=True
            )
            if grp % 2 == 0:
                nc.scalar.activation(
                    out=osb[:, ooff : ooff + wd],
                    in_=py[:, 0:wd],
                    func=AF.Identity,
                    bias=badj[:, h : h + 1],
                    scale=wsb[:, h : h + 1],
                )
            else:
                nc.vector.tensor_scalar(
                    out=osb[:, ooff : ooff + wd],
                    in0=py[:, 0:wd],
                    scalar1=wsb[:, h : h + 1],
                    scalar2=badj[:, h : h + 1],
                    op0=OP.mult,
                    op1=OP.add,
                )
            if grp % 3 == 2:
                # HAM warm filler: keeps PE activity dense while affines and
                # DMA pace the loop, so real matmuls stay at full clock
                pw = psB.tile([P, 512], F32, tag="ps")
                nc.tensor.matmul(
                    pw, lhsT=WM, rhs=xt[:, 0:512], start=True, stop=True
                )
        nc.sync.dma_start(out[s, :, 0:HALF_A], osb_a[:, 0:HALF_A])
        nc.sync.dma_start(out[s, :, HALF_A:HW], osb_b[:, 0 : HW - HALF_A])


_BUILT = None


def _build():
    global _BUILT
    if _BUILT is not None:
        return _BUILT
    nc = bacc.Bacc(
        "TRN2",
        target_bir_lowering=False,
        debug=False,
        enable_asserts=False,
        num_devices=N_CORES,
    )
    x_d = nc.dram_tensor("x", [SLABS, P, HW], F32, kind="ExternalInput")
    w_d = nc.dram_tensor("w2", [2, P, 1], F32, kind="ExternalInput")
    b_d = nc.dram_tensor("b2", [2, P, 1], F32, kind="ExternalInput")
    i_d = nc.dram_tensor("i128", [P, P], F32, kind="ExternalInput")
    bd_d = nc.dram_tensor("bd128", [P, P], F32, kind="ExternalInput")
    bdm_d = nc.dram_tensor("bdm128", [P, P], F32, kind="ExternalInput")
    o_d = nc.dram_tensor("out", [SLABS, P, HW], F32, kind="ExternalOutput")
    from contextlib import ExitStack

    with tile.TileContext(nc) as tc, ExitStack() as ctx:
        _emit(
            ctx, tc, x_d.ap(), w_d.ap(), b_d.ap(), i_d.ap(), bd_d.ap(),
            bdm_d.ap(), o_d.ap(),
        )
    nc.compile()
    _BUILT = nc
    return nc


def kernel(x, weight, bias, trace=False, tmpdir=None):
    x = np.ascontiguousarray(np.asarray(x, dtype=np.float32))
    weight = np.asarray(weight, dtype=np.float32)
    bias = np.asarray(bias, dtype=np.float32)
    assert x.shape == (N, C, H, W)

    nc = _build()

    w2 = np.ascontiguousarray(weight.reshape(2, P, 1))
    b2 = np.ascontiguousarray(bias.reshape(2, P, 1))
    i128 = np.eye(P, dtype=np.float32)
    idx = np.arange(P)
    bd128 = (idx[:, None] % G == idx[None, :] % G).astype(np.float32)
    bdm128 = (idx[:, None] // G == idx[None, :] // G).astype(np.float32)

    xs = x.reshape(N_CORES, SLABS, P, HW)
    in_maps = [
        {
            "x": xs[c], "w2": w2, "b2": b2, "i128": i128, "bd128": bd128,
            "bdm128": bdm128,
        }
        for c in range(N_CORES)
    ]
    res = bass_utils.run_bass_kernel_spmd(
        nc, in_maps, core_ids=list(range(N_CORES)), trace=trace, tmpdir=tmpdir
    )
    out = np.concatenate(
        [r["out"].reshape(1, N // N_CORES, C, H, W) for r in res.results], axis=0
    ).reshape(N, C, H, W)
    if trace:
        return out, res
    return out


# revision 48
# speedup vs baseline: 1.0921x; 1.0006x over previous
"""Trainium2 Bass kernel for BatchGroupItN (iterative whitening group norm).

Math (reference):
    x: (N=64, C=256, H=56, W=56) fp32.  Group of channel c is g = c % 32.
    xg[g, m] collects all elements with c % 32 == g  (m = 512*3136 per group).
    sigma = cov(xg) + eps*I  (32x32); wm = sigma^{-1/2} via 5 Newton-Schulz
    iters on trace-normalized sigma; out = (wm @ (xg - mu)) scattered back,
    then * weight + bias.

Strategy (8 cores, data-parallel over batch N):
    Each core owns 8 batches = 16 contiguous slabs of [128 channels, 3136 hw].
    Channel partition p of a slab belongs to group p % 32.
    Pass 1 (bf16): cast each slab to bf16, PE-transpose [128,128] chunks ->
    T [m,c] tiles, Gram matmuls accumulate S128 = sum T^T T in PSUM; the four
    32x32 diagonal blocks of S128 sum to S = sum x x^T.  Channel sums come
    from an in-place fp32 ACT copy with accum_out (full precision).
    Fold S128/sums to 32-wide via selector matmuls, AllReduce a packed
    [32,64] buffer across the 8 cores, then every core runs the (tiny)
    Newton-Schulz iterations and builds a block-diagonal WM = diag(wm x4).
    Pass 2 (fp32): y = WM @ x per [128,512] chunk on the PE, then one
    per-partition affine (scale=weight, bias=bias - wm@mu * weight,
    alternating DVE/ACT) and DMA out.  The first RESIDENT slabs stay in
    SBUF between passes; the rest are re-read from HBM.
"""

import numpy as np

import concourse.bass as bass
import concourse.bacc as bacc
import concourse.tile as tile
from concourse import bass_utils, mybir

F32 = mybir.dt.float32
F32R = mybir.dt.float32r
BF16 = mybir.dt.bfloat16
AX = mybir.AxisListType
OP = mybir.AluOpType
AF = mybir.ActivationFunctionType

N_CORES = 8
G = 32
T_ITERS = 5
EPS = 1e-5
N, C, H, W = 64, 256, 56, 56
HW = H * W  # 3136
P = 128
SLABS = 16  # per core: 8 batches x 2 channel-halves of 128
M_TOTAL = float(N * (C // G) * HW)  # 1,605,632 elements per group
RESIDENT = 10  # slabs kept in SBUF between pass 1 and pass 2
GRPS = (HW + 511) // 512  # 7: six full 512 groups + one 64 tail
N_WARM = 24  # dummy matmuls keeping the PE warm through the all-reduce


def _emit(ctx, tc, x, w2, b2, i128, i128r, bd, bdm, out):
    nc = tc.nc

    consts = ctx.enter_context(tc.tile_pool(name="consts", bufs=1))
    single = ctx.enter_context(tc.tile_pool(name="single", bufs=1))
    ns = ctx.enter_context(tc.tile_pool(name="ns", bufs=3))
    xres = ctx.enter_context(tc.tile_pool(name="xres", bufs=RESIDENT))
    xstream = ctx.enter_context(tc.tile_pool(name="xstream", bufs=2))
    xbp = ctx.enter_context(tc.tile_pool(name="xbp", bufs=2))
    tp = ctx.enter_context(tc.tile_pool(name="tp", bufs=3))
    outp = ctx.enter_context(tc.tile_pool(name="outp", bufs=4))
    psA = ctx.enter_context(tc.tile_pool(name="psA", bufs=1, space="PSUM"))
    # one shared 6-deep psum pool: pass-1 transpose staging and pass-2 matmul
    # outputs use the same slots (tag "ps"), so both passes pipeline 6 deep
    psB = ctx.enter_context(tc.tile_pool(name="psB", bufs=6, space="PSUM"))
    psS = ctx.enter_context(tc.tile_pool(name="psS", bufs=1, space="PSUM"))
    dram = ctx.enter_context(tc.tile_pool(name="dram", bufs=1, space="DRAM"))

    I128 = consts.tile([P, P], F32)
    nc.sync.dma_start(I128, i128)
    I128b = consts.tile([P, P], BF16)
    nc.vector.tensor_copy(I128b, I128)
    BD = consts.tile([P, P], F32)
    nc.sync.dma_start(BD, bd)
    BDM = consts.tile([P, P], F32)
    nc.sync.dma_start(BDM, bdm)
    I128r = consts.tile([P, P], F32R)
    nc.sync.dma_start(I128r, i128r)
    I32 = I128[0:G, 0:G]
    ones = consts.tile([P, G], F32)
    nc.vector.memset(ones, 1.0)
    wsb = consts.tile([P, 2], F32)
    bsb = consts.tile([P, 2], F32)
    for h in range(2):
        nc.sync.dma_start(wsb[:, h : h + 1], w2[h])
        nc.sync.dma_start(bsb[:, h : h + 1], b2[h])
    negepsI = consts.tile([G, G], F32)
    nc.vector.tensor_scalar_mul(negepsI, I32, -EPS)
    pack = single.tile([G, 64], F32)
    nc.vector.memset(pack, 0.0)

    # ---------------- pass 1: statistics (bf16 compute) ---------
    # psum_S cols 0:128 accumulate S128 = sum T^T T; col 128 accumulates the
    # channel sums (each Gram's rhs is [T_chunk | ones], one extra column).
    psum_S = psA.tile([P, 136], F32, tag="pS")

    xt_tiles = [None] * SLABS
    n_grams = SLABS * 25  # 6 groups x 4 chunks + 1 tail chunk, per slab
    gram_i = 0
    copy_i = 0
    for s in range(SLABS):
        if s < RESIDENT:
            xt = xres.tile([P, HW], F32R, tag="xr")
        else:
            xt = xstream.tile([P, HW], F32R, tag="xs")
        xt_tiles[s] = xt
        nc.sync.dma_start(xt, x[s])
        xb = xbp.tile([P, HW], BF16, tag="xb")
        nc.vector.tensor_copy(xb, xt.bitcast(F32))  # fp32 -> bf16 cast
        for grp in range(GRPS):
            off = 512 * grp
            wd = min(512, HW - off)  # 512 or 64
            nch = (wd + 127) // 128  # 4 or 1
            pt = psB.tile([P, 512], BF16, tag="ps")
            for k in range(nch):
                cw = min(128, wd - 128 * k)  # 128 or 64
                nc.tensor.transpose(
                    pt[0:cw, 128 * k : 128 * k + P],
                    xb[:, off + 128 * k : off + 128 * k + cw],
                    I128b,
                )
            # tsb chunk k occupies [:, k, 0:128]; [:, k, 128] is a ones
            # column so the Gram rhs [T_k | 1] yields channel sums for free
            tsb = tp.tile([P, 4, 132], BF16, tag="tsb")
            copy_i += 1
            eng = nc.vector if copy_i % 2 == 0 else nc.scalar
            if wd == 512:
                if eng is nc.vector:
                    nc.vector.tensor_copy(tsb[:, :, 0:P], pt)
                else:
                    nc.scalar.copy(tsb[:, :, 0:P], pt)
                nc.vector.memset(tsb[:, :, P : P + 1], 1.0)
            else:
                if eng is nc.vector:
                    nc.vector.tensor_copy(tsb[0:wd, 0, 0:P], pt[0:wd, 0:P])
                else:
                    nc.scalar.copy(tsb[0:wd, 0, 0:P], pt[0:wd, 0:P])
                nc.vector.memset(tsb[0:wd, 0, P : P + 1], 1.0)
            for k in range(nch):
                cw = min(128, wd - 128 * k)
                gram_i += 1
                nc.tensor.matmul(
                    psum_S[:, 0 : P + 1],
                    lhsT=tsb[0:cw, k, 0:P],
                    rhs=tsb[0:cw, k, 0 : P + 1],
                    start=(gram_i == 1),
                    stop=(gram_i == n_grams),
                )

    # ---------------- fold + all-reduce ----------------
    Ssb = single.tile([P, 136], F32)
    nc.vector.tensor_copy(Ssb[:, 0 : P + 1], psum_S[:, 0 : P + 1])
    sums128 = Ssb[:, P : P + 1]
    ps32 = psS.tile([G, 64], F32, tag="sps")
    for i in range(4):
        # lhsT = columns of I128: selects ONLY row-block i of S128
        nc.tensor.matmul(
            ps32[:, 0:G],
            lhsT=I128[:, G * i : G * i + G],
            rhs=Ssb[:, G * i : G * i + G],
            start=(i == 0),
            stop=(i == 3),
        )
    nc.tensor.matmul(ps32[:, G : G + 1], lhsT=BD[:, 0:G], rhs=sums128, start=True, stop=True)
    nc.vector.tensor_copy(pack[:, 0 : G + 1], ps32[:, 0 : G + 1])

    cc_in = dram.tile([G, 64], F32)
    cc_out = dram.tile([G, 64], F32)
    nc.sync.dma_start(cc_in, pack)
    nc.gpsimd.collective_compute(
        "AllReduce",
        OP.add,
        replica_groups=[list(range(N_CORES))],
        ins=[cc_in.opt()],
        outs=[cc_out.opt()],
    )

    # keep the PE's HAM clock warm through the all-reduce wait: identity
    # matmuls on already-resident data into otherwise-idle psum banks
    warm_src = xt_tiles[0]
    for wi in range(8):
        pw = psB.tile([P, 512], F32, tag="ps")
        nc.tensor.matmul(
            pw, lhsT=I128r, rhs=warm_src[:, 0:512], start=True, stop=True
        )

    packr = single.tile([G, 64], F32)
    packr_dma = nc.sync.dma_start(packr, cc_out)

    # post-all-reduce warm-up: depends on packr, so these run right after the
    # collective lands, re-warming the PE while the stats chain executes
    from concourse.tile import add_dep_helper

    for wi in range(8):
        pw = psB.tile([P, 512], F32, tag="ps")
        mm = nc.tensor.matmul(
            pw, lhsT=I128r, rhs=warm_src[:, 0:512], start=True, stop=True
        )
        add_dep_helper(mm.ins, packr_dma.ins, sync=True, reason="run after AR")

    # ---------------- sigma, trace, Newton-Schulz ----------------
    # Rescaled NS iteration: with P_k = 1.5^k Q_k,
    #   Q_{k+1} = Q_k - Q_k^3 (0.5 * 1.5^(2k-1) * sigma_N),  Q_0 = I
    # and wm = 1.5^5 Q_5 sqrt(tinv), folded as sqrt(1.5^10 * tinv).
    inv_m = 1.0 / M_TOTAL
    mu = single.tile([G, 1], F32)
    nc.vector.tensor_scalar_mul(mu, packr[:, G : G + 1], inv_m)
    ps_mr = psS.tile([1, G], F32, tag="sps")
    nc.tensor.transpose(ps_mr, mu, I32)
    murow = single.tile([1, G], F32)
    nc.vector.tensor_copy(murow, ps_mr)
    # ps_mm = mu mu^T - eps*I  (second accumulating matmul adds -eps*I)
    ps_mm = psS.tile([G, G], F32, tag="sps")
    nc.tensor.matmul(ps_mm, lhsT=murow, rhs=murow, start=True, stop=False)
    nc.tensor.matmul(ps_mm, lhsT=negepsI, rhs=I32, start=False, stop=True)
    sigma = single.tile([G, G], F32)
    nc.vector.tensor_scalar_mul(sigma, packr[:, 0:G], inv_m)
    nc.vector.tensor_sub(sigma, sigma, ps_mm)

    diag = single.tile([G, G], F32)
    nc.vector.tensor_mul(diag, sigma, I32)
    dcol = single.tile([G, 1], F32)
    nc.vector.tensor_reduce(dcol, diag, AX.X, OP.add)
    ps_tr = psS.tile([1, 1], F32, tag="sps")
    nc.tensor.matmul(ps_tr, lhsT=dcol, rhs=ones[0:G, 0:1], start=True, stop=True)
    trsb = single.tile([1, 1], F32)
    nc.vector.tensor_copy(trsb, ps_tr)
    tinv = single.tile([1, 1], F32)
    nc.vector.reciprocal(tinv, trsb)
    ps_b32 = psS.tile([G, 1], F32, tag="sps")
    nc.tensor.matmul(ps_b32, lhsT=ones[0:1, 0:G], rhs=tinv, start=True, stop=True)
    tinv32 = single.tile([G, 1], F32)
    nc.vector.tensor_copy(tinv32, ps_b32)
    # stinv = sqrt(1.5^10 * tinv)  (per-partition broadcast)
    tinv57 = single.tile([G, 1], F32)
    nc.vector.tensor_scalar_mul(tinv57, tinv32, 1.5**10)
    stinv = single.tile([G, 1], F32)
    nc.scalar.sqrt(stinv, tinv57)
    # pre-scaled sigma_k = sigma * tinv * (0.5 * 1.5^(2k-1))
    sigks = []
    for k in range(T_ITERS):
        sk = single.tile([G, G], F32, tag=f"sigk{k}")
        nc.vector.tensor_scalar(
            out=sk, in0=sigma, scalar1=tinv32, scalar2=0.5 * 1.5 ** (2 * k - 1),
            op0=OP.mult, op1=OP.mult,
        )
        sigks.append(sk)

    # NS loop with 4 cross-engine transitions per iteration:
    #   PE: A_ps = Q^2, B_ps = Q sig_k (independent, back-to-back)
    #   DVE: copy A, B to SBUF;  PE: C_ps = A^T B = Q^3 sig_k;  DVE: Q -= C
    Pcur = single.tile([G, G], F32, tag="P0")
    nc.vector.tensor_copy(Pcur, I32)
    wi = 0
    for k in range(T_ITERS):
        psa = psS.tile([G, G], F32, tag="sps")
        nc.tensor.matmul(psa, lhsT=Pcur, rhs=Pcur, start=True, stop=True)
        psb_ = psB.tile([G, G], F32, tag="ps")
        nc.tensor.matmul(psb_, lhsT=Pcur, rhs=sigks[k], start=True, stop=True)
        asb = ns.tile([G, G], F32, tag="nsA")
        nc.vector.tensor_copy(asb, psa)
        bsb_ = ns.tile([G, G], F32, tag="nsB")
        nc.vector.tensor_copy(bsb_, psb_)
        # keep the PE's HAM clock warm while DVE runs the copies
        for _ in range(2):
            pw = psB.tile([P, 512], F32, tag="ps")
            mm = nc.tensor.matmul(
                pw, lhsT=I128r, rhs=warm_src[:, 0:512], start=True, stop=True
            )
            add_dep_helper(mm.ins, packr_dma.ins, sync=True, reason="run after AR")
        psc = psS.tile([G, G], F32, tag="sps")
        nc.tensor.matmul(psc, lhsT=asb, rhs=bsb_, start=True, stop=True)
        pn = ns.tile([G, G], F32, tag="nsP")
        nc.vector.tensor_sub(pn, Pcur, psc)
        Pcur = pn

    wm = single.tile([G, G], F32)
    nc.vector.tensor_scalar_mul(wm, Pcur, stinv)

    # block-diagonal WM = diag(wm, wm, wm, wm): write the four diagonal
    # blocks on the PE, then one masked copy (off-diag psum garbage is
    # finite leftovers from the warm-up matmuls, zeroed by the mask)
    ps_wm = psB.tile([P, P], F32, tag="ps")
    for i in range(4):
        nc.tensor.matmul(
            ps_wm[G * i : G * i + G, G * i : G * i + G],
            lhsT=wm,
            rhs=I32,
            start=True,
            stop=True,
            tile_position=(0, G * i),
        )
    WM = single.tile([P, P], F32R)
    nc.vector.tensor_mul(WM, ps_wm, BDM)

    # per-partition affine: scale = weight, bias = bias - (wm @ mu) * weight
    ps_v = psS.tile([G, 1], F32, tag="sps")
    nc.tensor.matmul(ps_v, lhsT=wm, rhs=mu, start=True, stop=True)
    vsb = single.tile([G, 1], F32)
    nc.vector.tensor_copy(vsb, ps_v)
    ps_v128 = psS.tile([P, 1], F32, tag="sps")
    nc.tensor.matmul(ps_v128, lhsT=BD[0:G, :], rhs=vsb, start=True, stop=True)
    v128 = single.tile([P, 1], F32)
    nc.vector.tensor_copy(v128, ps_v128)
    badj = single.tile([P, 2], F32)
    for h in range(2):
        nc.vector.tensor_mul(badj[:, h : h + 1], v128, wsb[:, h : h + 1])
        nc.vector.tensor_sub(badj[:, h : h + 1], bsb[:, h : h + 1], badj[:, h : h + 1])

    # ---------------- pass 2: normalize (fp32) ----------------
    # order: last two streamed slabs first (still resident in xstream slots);
    # re-read slabs spaced every third so write+read HBM demand stays under
    # the per-core bandwidth
    order = [14, 15, 0, 10, 1, 2, 11, 3, 4, 12, 5, 6, 13, 7, 8, 9]
    HALF_A = 1536  # chunks 0..2; chunks 3..6 cover 1536:3136 (1600 cols)
    for s in order:
        if RESIDENT <= s < SLABS - 2:
            xt = xstream.tile([P, HW], F32R, tag="xs")
            nc.sync.dma_start(xt, x[s])
        else:
            xt = xt_tiles[s]
        h = s % 2
        osb_a = outp.tile([P, HW - HALF_A], F32, tag="osb")
        osb_b = outp.tile([P, HW - HALF_A], F32, tag="osb")
        for grp in range(GRPS):
            off = 512 * grp
            wd = min(512, HW - off)
            osb, ooff = (osb_a, off) if off < HALF_A else (osb_b, off - HALF_A)
            py = psB.tile([P, 512], F32, tag="ps")
            # float32r: single-pass PE matmul (plain fp32 lowers to two
            # half-rate passes); ~1e-4-class rounding on the whitening
            # product only, statistics are unaffected
            nc.tensor.matmul(
                py[:, 0:wd],
                lhsT=WM,
                rhs=xt[:, off : off + wd],
                start=True,
                stop=True,
            )
            if grp % 2 == 0:
                nc.scalar.activation(
                    out=osb[:, ooff : ooff + wd],
                    in_=py[:, 0:wd],
                    func=AF.Identity,
                    bias=badj[:, h : h + 1],
                    scale=wsb[:, h : h + 1],
                )
            else:
                nc.vector.tensor_scalar(
                    out=osb[:, ooff : ooff + wd],
                    in0=py[:, 0:wd],
                    scalar1=wsb[:, h : h + 1],
                    scalar2=badj[:, h : h + 1],
                    op0=OP.mult,
                    op1=OP.add,
                )
        nc.sync.dma_start(out[s, :, 0:HALF_A], osb_a[:, 0:HALF_A])
        nc.sync.dma_start(out[s, :, HALF_A:HW], osb_b[:, 0 : HW - HALF_A])


_BUILT = None


def _build():
    global _BUILT
    if _BUILT is not None:
        return _BUILT
    nc = bacc.Bacc(
        "TRN2",
        target_bir_lowering=False,
        debug=False,
        enable_asserts=False,
        num_devices=N_CORES,
    )
    x_d = nc.dram_tensor("x", [SLABS, P, HW], F32R, kind="ExternalInput")
    w_d = nc.dram_tensor("w2", [2, P, 1], F32, kind="ExternalInput")
    b_d = nc.dram_tensor("b2", [2, P, 1], F32, kind="ExternalInput")
    i_d = nc.dram_tensor("i128", [P, P], F32, kind="ExternalInput")
    ir_d = nc.dram_tensor("i128r", [P, P], F32R, kind="ExternalInput")
    bd_d = nc.dram_tensor("bd128", [P, P], F32, kind="ExternalInput")
    bdm_d = nc.dram_tensor("bdm128", [P, P], F32, kind="ExternalInput")
    o_d = nc.dram_tensor("out", [SLABS, P, HW], F32, kind="ExternalOutput")
    from contextlib import ExitStack

    with tile.TileContext(nc) as tc, ExitStack() as ctx:
        _emit(
            ctx, tc, x_d.ap(), w_d.ap(), b_d.ap(), i_d.ap(), ir_d.ap(), bd_d.ap(),
            bdm_d.ap(), o_d.ap(),
        )
    nc.compile()
    _BUILT = nc
    return nc


def kernel(x, weight, bias, trace=False, tmpdir=None):
    x = np.ascontiguousarray(np.asarray(x, dtype=np.float32))
    weight = np.asarray(weight, dtype=np.float32)
    bias = np.asarray(bias, dtype=np.float32)
    assert x.shape == (N, C, H, W)

    nc = _build()

    w2 = np.ascontiguousarray(weight.reshape(2, P, 1))
    b2 = np.ascontiguousarray(bias.reshape(2, P, 1))
    i128 = np.eye(P, dtype=np.float32)
    idx = np.arange(P)
    bd128 = (idx[:, None] % G == idx[None, :] % G).astype(np.float32)
    bdm128 = (idx[:, None] // G == idx[None, :] // G).astype(np.float32)

    xs = x.reshape(N_CORES, SLABS, P, HW)
    in_maps = [
        {
            "x": xs[c], "w2": w2, "b2": b2, "i128": i128, "i128r": i128,
            "bd128": bd128, "bdm128": bdm128,
        }
        for c in range(N_CORES)
    ]
    res = bass_utils.run_bass_kernel_spmd(
        nc, in_maps, core_ids=list(range(N_CORES)), trace=trace, tmpdir=tmpdir
    )
    out = np.concatenate(
        [r["out"].reshape(1, N // N_CORES, C, H, W) for r in res.results], axis=0
    ).reshape(N, C, H, W)
    if trace:
        return out, res
    return out


# revision 49
# speedup vs baseline: 1.3557x; 1.2414x over previous
"""Trainium2 Bass kernel for BatchGroupItN (iterative whitening group norm).

Math (reference):
    x: (N=64, C=256, H=56, W=56) fp32.  Group of channel c is g = c % 32.
    xg[g, m] collects all elements with c % 32 == g  (m = 512*3136 per group).
    sigma = cov(xg) + eps*I  (32x32); wm = sigma^{-1/2} via 5 Newton-Schulz
    iters on trace-normalized sigma; out = (wm @ (xg - mu)) scattered back,
    then * weight + bias.

Strategy (8 cores, data-parallel over batch N):
    Each core owns 8 batches = 16 contiguous slabs of [128 channels, 3136 hw].
    Channel partition p of a slab belongs to group p % 32.
    Pass 1 (bf16): cast each slab to bf16, PE-transpose [128,128] chunks ->
    T [m,c] tiles, Gram matmuls accumulate S128 = sum T^T T in PSUM; the four
    32x32 diagonal blocks of S128 sum to S = sum x x^T.  Channel sums come
    from an in-place fp32 ACT copy with accum_out (full precision).
    Fold S128/sums to 32-wide via selector matmuls, AllReduce a packed
    [32,64] buffer across the 8 cores, then every core runs the (tiny)
    Newton-Schulz iterations and builds a block-diagonal WM = diag(wm x4).
    Pass 2 (fp32): y = WM @ x per [128,512] chunk on the PE, then one
    per-partition affine (scale=weight, bias=bias - wm@mu * weight,
    alternating DVE/ACT) and DMA out.  The first RESIDENT slabs stay in
    SBUF between passes; the rest are re-read from HBM.
"""

import numpy as np

import concourse.bass as bass
import concourse.bacc as bacc
import concourse.tile as tile
from concourse import bass_utils, mybir

F32 = mybir.dt.float32
F32R = mybir.dt.float32r
BF16 = mybir.dt.bfloat16
AX = mybir.AxisListType
OP = mybir.AluOpType
AF = mybir.ActivationFunctionType

N_CORES = 8
G = 32
T_ITERS = 5
EPS = 1e-5
N, C, H, W = 64, 256, 56, 56
HW = H * W  # 3136
P = 128
SLABS = 16  # per core: 8 batches x 2 channel-halves of 128
M_TOTAL = float(N * (C // G) * HW)  # 1,605,632 elements per group
RESIDENT = 10  # slabs kept in SBUF between pass 1 and pass 2
GRPS = (HW + 511) // 512  # 7: six full 512 groups + one 64 tail
N_WARM = 24  # dummy matmuls keeping the PE warm through the all-reduce


def _emit(ctx, tc, x, w2, b2, i128, i128r, bd, bdm, out):
    nc = tc.nc

    consts = ctx.enter_context(tc.tile_pool(name="consts", bufs=1))
    single = ctx.enter_context(tc.tile_pool(name="single", bufs=1))
    ns = ctx.enter_context(tc.tile_pool(name="ns", bufs=3))
    xres = ctx.enter_context(tc.tile_pool(name="xres", bufs=RESIDENT))
    xstream = ctx.enter_context(tc.tile_pool(name="xstream", bufs=3))
    xbp = ctx.enter_context(tc.tile_pool(name="xbp", bufs=2))
    tp = ctx.enter_context(tc.tile_pool(name="tp", bufs=3))
    outp = ctx.enter_context(tc.tile_pool(name="outp", bufs=3))
    psA = ctx.enter_context(tc.tile_pool(name="psA", bufs=1, space="PSUM"))
    # one shared 6-deep psum pool: pass-1 transpose staging and pass-2 matmul
    # outputs use the same slots (tag "ps"), so both passes pipeline 6 deep
    psB = ctx.enter_context(tc.tile_pool(name="psB", bufs=6, space="PSUM"))
    psS = ctx.enter_context(tc.tile_pool(name="psS", bufs=1, space="PSUM"))
    dram = ctx.enter_context(tc.tile_pool(name="dram", bufs=1, space="DRAM"))

    I128 = consts.tile([P, P], F32)
    nc.sync.dma_start(I128, i128)
    I128b = consts.tile([P, P], BF16)
    nc.vector.tensor_copy(I128b, I128)
    BD = consts.tile([P, P], F32)
    nc.sync.dma_start(BD, bd)
    BDM = consts.tile([P, P], F32)
    nc.sync.dma_start(BDM, bdm)
    I128r = consts.tile([P, P], F32R)
    nc.sync.dma_start(I128r, i128r)
    I32 = I128[0:G, 0:G]
    ones = consts.tile([P, G], F32)
    nc.vector.memset(ones, 1.0)
    wsb = consts.tile([P, 2], F32)
    bsb = consts.tile([P, 2], F32)
    for h in range(2):
        nc.sync.dma_start(wsb[:, h : h + 1], w2[h])
        nc.sync.dma_start(bsb[:, h : h + 1], b2[h])
    negepsI = consts.tile([G, G], F32)
    nc.vector.tensor_scalar_mul(negepsI, I32, -EPS)
    pack = single.tile([G, 64], F32)
    nc.vector.memset(pack, 0.0)

    # ---------------- pass 1: statistics (bf16 compute) ---------
    # psum_S cols 0:128 accumulate S128 = sum T^T T; col 128 accumulates the
    # channel sums (each Gram's rhs is [T_chunk | ones], one extra column).
    psum_S = psA.tile([P, 136], F32, tag="pS")

    xt_tiles = [None] * SLABS
    n_grams = SLABS * 25  # 6 groups x 4 chunks + 1 tail chunk, per slab
    gram_i = 0
    copy_i = 0
    for s in range(SLABS):
        if s < RESIDENT:
            xt = xres.tile([P, HW], F32R, tag="xr")
        else:
            xt = xstream.tile([P, HW], F32R, tag="xs")
        xt_tiles[s] = xt
        nc.sync.dma_start(xt, x[s])
        xb = xbp.tile([P, HW], BF16, tag="xb")
        nc.vector.tensor_copy(xb, xt.bitcast(F32))  # fp32 -> bf16 cast
        for grp in range(GRPS):
            off = 512 * grp
            wd = min(512, HW - off)  # 512 or 64
            nch = (wd + 127) // 128  # 4 or 1
            pt = psB.tile([P, 512], BF16, tag="ps")
            for k in range(nch):
                cw = min(128, wd - 128 * k)  # 128 or 64
                nc.tensor.transpose(
                    pt[0:cw, 128 * k : 128 * k + P],
                    xb[:, off + 128 * k : off + 128 * k + cw],
                    I128b,
                )
            # tsb chunk k occupies [:, k, 0:128]; [:, k, 128] is a ones
            # column so the Gram rhs [T_k | 1] yields channel sums for free
            tsb = tp.tile([P, 4, 132], BF16, tag="tsb")
            copy_i += 1
            eng = nc.vector if copy_i % 2 == 0 else nc.scalar
            if wd == 512:
                if eng is nc.vector:
                    nc.vector.tensor_copy(tsb[:, :, 0:P], pt)
                else:
                    nc.scalar.copy(tsb[:, :, 0:P], pt)
                nc.vector.memset(tsb[:, :, P : P + 1], 1.0)
            else:
                if eng is nc.vector:
                    nc.vector.tensor_copy(tsb[0:wd, 0, 0:P], pt[0:wd, 0:P])
                else:
                    nc.scalar.copy(tsb[0:wd, 0, 0:P], pt[0:wd, 0:P])
                nc.vector.memset(tsb[0:wd, 0, P : P + 1], 1.0)
            for k in range(nch):
                cw = min(128, wd - 128 * k)
                gram_i += 1
                nc.tensor.matmul(
                    psum_S[:, 0 : P + 1],
                    lhsT=tsb[0:cw, k, 0:P],
                    rhs=tsb[0:cw, k, 0 : P + 1],
                    start=(gram_i == 1),
                    stop=(gram_i == n_grams),
                )

    # ---------------- fold + all-reduce ----------------
    Ssb = single.tile([P, 136], F32)
    nc.vector.tensor_copy(Ssb[:, 0 : P + 1], psum_S[:, 0 : P + 1])
    sums128 = Ssb[:, P : P + 1]
    ps32 = psS.tile([G, 64], F32, tag="sps")
    for i in range(4):
        # lhsT = columns of I128: selects ONLY row-block i of S128
        nc.tensor.matmul(
            ps32[:, 0:G],
            lhsT=I128[:, G * i : G * i + G],
            rhs=Ssb[:, G * i : G * i + G],
            start=(i == 0),
            stop=(i == 3),
        )
    nc.tensor.matmul(ps32[:, G : G + 1], lhsT=BD[:, 0:G], rhs=sums128, start=True, stop=True)
    nc.vector.tensor_copy(pack[:, 0 : G + 1], ps32[:, 0 : G + 1])

    cc_in = dram.tile([G, 64], F32)
    cc_out = dram.tile([G, 64], F32)
    nc.sync.dma_start(cc_in, pack)
    nc.gpsimd.collective_compute(
        "AllReduce",
        OP.add,
        replica_groups=[list(range(N_CORES))],
        ins=[cc_in.opt()],
        outs=[cc_out.opt()],
    )

    # keep the PE's HAM clock warm through the all-reduce wait: identity
    # matmuls on already-resident data into otherwise-idle psum banks
    warm_src = xt_tiles[0]
    for wi in range(8):
        pw = psB.tile([P, 512], F32, tag="ps")
        nc.tensor.matmul(
            pw, lhsT=I128r, rhs=warm_src[:, 0:512], start=True, stop=True
        )

    packr = single.tile([G, 64], F32)
    packr_dma = nc.sync.dma_start(packr, cc_out)

    # post-all-reduce warm-up: depends on packr, so these run right after the
    # collective lands, re-warming the PE while the stats chain executes
    from concourse.tile import add_dep_helper

    for wi in range(8):
        pw = psB.tile([P, 512], F32, tag="ps")
        mm = nc.tensor.matmul(
            pw, lhsT=I128r, rhs=warm_src[:, 0:512], start=True, stop=True
        )
        add_dep_helper(mm.ins, packr_dma.ins, sync=True, reason="run after AR")

    # ---------------- sigma, trace, Newton-Schulz ----------------
    # Rescaled NS iteration: with P_k = 1.5^k Q_k,
    #   Q_{k+1} = Q_k - Q_k^3 (0.5 * 1.5^(2k-1) * sigma_N),  Q_0 = I
    # and wm = 1.5^5 Q_5 sqrt(tinv), folded as sqrt(1.5^10 * tinv).
    inv_m = 1.0 / M_TOTAL
    mu = single.tile([G, 1], F32)
    nc.vector.tensor_scalar_mul(mu, packr[:, G : G + 1], inv_m)
    ps_mr = psS.tile([1, G], F32, tag="sps")
    nc.tensor.transpose(ps_mr, mu, I32)
    murow = single.tile([1, G], F32)
    nc.vector.tensor_copy(murow, ps_mr)
    # ps_mm = mu mu^T - eps*I  (second accumulating matmul adds -eps*I)
    ps_mm = psS.tile([G, G], F32, tag="sps")
    nc.tensor.matmul(ps_mm, lhsT=murow, rhs=murow, start=True, stop=False)
    nc.tensor.matmul(ps_mm, lhsT=negepsI, rhs=I32, start=False, stop=True)
    sigma = single.tile([G, G], F32)
    nc.vector.tensor_scalar_mul(sigma, packr[:, 0:G], inv_m)
    nc.vector.tensor_sub(sigma, sigma, ps_mm)

    diag = single.tile([G, G], F32)
    nc.vector.tensor_mul(diag, sigma, I32)
    dcol = single.tile([G, 1], F32)
    nc.vector.tensor_reduce(dcol, diag, AX.X, OP.add)
    ps_tr = psS.tile([1, 1], F32, tag="sps")
    nc.tensor.matmul(ps_tr, lhsT=dcol, rhs=ones[0:G, 0:1], start=True, stop=True)
    trsb = single.tile([1, 1], F32)
    nc.vector.tensor_copy(trsb, ps_tr)
    tinv = single.tile([1, 1], F32)
    nc.vector.reciprocal(tinv, trsb)
    ps_b32 = psS.tile([G, 1], F32, tag="sps")
    nc.tensor.matmul(ps_b32, lhsT=ones[0:1, 0:G], rhs=tinv, start=True, stop=True)
    tinv32 = single.tile([G, 1], F32)
    nc.vector.tensor_copy(tinv32, ps_b32)
    # stinv = sqrt(1.5^10 * tinv)  (per-partition broadcast)
    tinv57 = single.tile([G, 1], F32)
    nc.vector.tensor_scalar_mul(tinv57, tinv32, 1.5**10)
    stinv = single.tile([G, 1], F32)
    nc.scalar.sqrt(stinv, tinv57)
    # pre-scaled sigma_k = sigma * tinv * (0.5 * 1.5^(2k-1))
    sigks = []
    for k in range(T_ITERS):
        sk = single.tile([G, G], F32, tag=f"sigk{k}")
        nc.vector.tensor_scalar(
            out=sk, in0=sigma, scalar1=tinv32, scalar2=0.5 * 1.5 ** (2 * k - 1),
            op0=OP.mult, op1=OP.mult,
        )
        sigks.append(sk)

    # NS loop with 4 cross-engine transitions per iteration:
    #   PE: A_ps = Q^2, B_ps = Q sig_k (independent, back-to-back)
    #   DVE: copy A, B to SBUF;  PE: C_ps = A^T B = Q^3 sig_k;  DVE: Q -= C
    Pcur = single.tile([G, G], F32, tag="P0")
    nc.vector.tensor_copy(Pcur, I32)
    wi = 0
    for k in range(T_ITERS):
        psa = psS.tile([G, G], F32, tag="sps")
        nc.tensor.matmul(psa, lhsT=Pcur, rhs=Pcur, start=True, stop=True)
        psb_ = psB.tile([G, G], F32, tag="ps")
        nc.tensor.matmul(psb_, lhsT=Pcur, rhs=sigks[k], start=True, stop=True)
        asb = ns.tile([G, G], F32, tag="nsA")
        nc.vector.tensor_copy(asb, psa)
        bsb_ = ns.tile([G, G], F32, tag="nsB")
        nc.vector.tensor_copy(bsb_, psb_)
        # keep the PE's HAM clock warm while DVE runs the copies
        for _ in range(2):
            pw = psB.tile([P, 512], F32, tag="ps")
            mm = nc.tensor.matmul(
                pw, lhsT=I128r, rhs=warm_src[:, 0:512], start=True, stop=True
            )
            add_dep_helper(mm.ins, packr_dma.ins, sync=True, reason="run after AR")
        psc = psS.tile([G, G], F32, tag="sps")
        nc.tensor.matmul(psc, lhsT=asb, rhs=bsb_, start=True, stop=True)
        pn = ns.tile([G, G], F32, tag="nsP")
        nc.vector.tensor_sub(pn, Pcur, psc)
        Pcur = pn

    wm = single.tile([G, G], F32)
    nc.vector.tensor_scalar_mul(wm, Pcur, stinv)

    # block-diagonal WM = diag(wm, wm, wm, wm): write the four diagonal
    # blocks on the PE, then one masked copy (off-diag psum garbage is
    # finite leftovers from the warm-up matmuls, zeroed by the mask)
    ps_wm = psB.tile([P, P], F32, tag="ps")
    for i in range(4):
        nc.tensor.matmul(
            ps_wm[G * i : G * i + G, G * i : G * i + G],
            lhsT=wm,
            rhs=I32,
            start=True,
            stop=True,
            tile_position=(0, G * i),
        )
    WM = single.tile([P, P], F32R)
    nc.vector.tensor_mul(WM, ps_wm, BDM)

    # per-partition affine: scale = weight, bias = bias - (wm @ mu) * weight
    ps_v = psS.tile([G, 1], F32, tag="sps")
    nc.tensor.matmul(ps_v, lhsT=wm, rhs=mu, start=True, stop=True)
    vsb = single.tile([G, 1], F32)
    nc.vector.tensor_copy(vsb, ps_v)
    ps_v128 = psS.tile([P, 1], F32, tag="sps")
    nc.tensor.matmul(ps_v128, lhsT=BD[0:G, :], rhs=vsb, start=True, stop=True)
    v128 = single.tile([P, 1], F32)
    nc.vector.tensor_copy(v128, ps_v128)
    badj = single.tile([P, 2], F32)
    for h in range(2):
        nc.vector.tensor_mul(badj[:, h : h + 1], v128, wsb[:, h : h + 1])
        nc.vector.tensor_sub(badj[:, h : h + 1], bsb[:, h : h + 1], badj[:, h : h + 1])

    # ---------------- pass 2: normalize (fp32) ----------------
    # order: last two streamed slabs first (still resident in xstream slots);
    # re-read slabs spaced every third so write+read HBM demand stays under
    # the per-core bandwidth
    order = [14, 15, 0, 10, 1, 2, 11, 3, 4, 12, 5, 6, 13, 7, 8, 9]
    HALF_A = 1536  # chunks 0..2; chunks 3..6 cover 1536:3136 (1600 cols)
    for s in order:
        if RESIDENT <= s < SLABS - 2:
            xt = xstream.tile([P, HW], F32R, tag="xs")
            nc.sync.dma_start(xt, x[s])
        else:
            xt = xt_tiles[s]
        h = s % 2
        osb_a = outp.tile([P, HW - HALF_A], F32, tag="osb")
        osb_b = outp.tile([P, HW - HALF_A], F32, tag="osb")
        for grp in range(GRPS):
            off = 512 * grp
            wd = min(512, HW - off)
            osb, ooff = (osb_a, off) if off < HALF_A else (osb_b, off - HALF_A)
            py = psB.tile([P, 512], F32, tag="ps")
            # float32r: single-pass PE matmul (plain fp32 lowers to two
            # half-rate passes); ~1e-4-class rounding on the whitening
            # product only, statistics are unaffected
            nc.tensor.matmul(
                py[:, 0:wd],
                lhsT=WM,
                rhs=xt[:, off : off + wd],
                start=True,
                stop=True,
            )
            if grp % 2 == 0:
                nc.scalar.activation(
                    out=osb[:, ooff : ooff + wd],
                    in_=py[:, 0:wd],
                    func=AF.Identity,
                    bias=badj[:, h : h + 1],
                    scale=wsb[:, h : h + 1],
                )
            else:
                nc.vector.tensor_scalar(
                    out=osb[:, ooff : ooff + wd],
                    in0=py[:, 0:wd],
                    scalar1=wsb[:, h : h + 1],
                    scalar2=badj[:, h : h + 1],
                    op0=OP.mult,
                    op1=OP.add,
                )
        nc.sync.dma_start(out[s, :, 0:HALF_A], osb_a[:, 0:HALF_A])
        nc.sync.dma_start(out[s, :, HALF_A:HW], osb_b[:, 0 : HW - HALF_A])


_BUILT = None


def _build():
    global _BUILT
    if _BUILT is not None:
        return _BUILT
    nc = bacc.Bacc(
        "TRN2",
        target_bir_lowering=False,
        debug=False,
        enable_asserts=False,
        num_devices=N_CORES,
    )
    x_d = nc.dram_tensor("x", [SLABS, P, HW], F32R, kind="ExternalInput")
    w_d = nc.dram_tensor("w2", [2, P, 1], F32, kind="ExternalInput")
    b_d = nc.dram_tensor("b2", [2, P, 1], F32, kind="ExternalInput")
    i_d = nc.dram_tensor("i128", [P, P], F32, kind="ExternalInput")
    ir_d = nc.dram_tensor("i128r", [P, P], F32R, kind="ExternalInput")
    bd_d = nc.dram_tensor("bd128", [P, P], F32, kind="ExternalInput")
    bdm_d = nc.dram_tensor("bdm128", [P, P], F32, kind="ExternalInput")
    o_d = nc.dram_tensor("out", [SLABS, P, HW], F32, kind="ExternalOutput")
    from contextlib import ExitStack

    with tile.TileContext(nc) as tc, ExitStack() as ctx:
        _emit(
            ctx, tc, x_d.ap(), w_d.ap(), b_d.ap(), i_d.ap(), ir_d.ap(), bd_d.ap(),
            bdm_d.ap(), o_d.ap(),
        )
    nc.compile()
    _BUILT = nc
    return nc


def kernel(x, weight, bias, trace=False, tmpdir=None):
    x = np.ascontiguousarray(np.asarray(x, dtype=np.float32))
    weight = np.asarray(weight, dtype=np.float32)
    bias = np.asarray(bias, dtype=np.float32)
    assert x.shape == (N, C, H, W)

    nc = _build()

    w2 = np.ascontiguousarray(weight.reshape(2, P, 1))
    b2 = np.ascontiguousarray(bias.reshape(2, P, 1))
    i128 = np.eye(P, dtype=np.float32)
    idx = np.arange(P)
    bd128 = (idx[:, None] % G == idx[None, :] % G).astype(np.float32)
    bdm128 = (idx[:, None] // G == idx[None, :] // G).astype(np.float32)

    xs = x.reshape(N_CORES, SLABS, P, HW)
    in_maps = [
        {
            "x": xs[c], "w2": w2, "b2": b2, "i128": i128, "i128r": i128,
            "bd128": bd128, "bdm128": bdm128,
        }
        for c in range(N_CORES)
    ]
    res = bass_utils.run_bass_kernel_spmd(
        nc, in_maps, core_ids=list(range(N_CORES)), trace=trace, tmpdir=tmpdir
    )
    out = np.concatenate(
        [r["out"].reshape(1, N // N_CORES, C, H, W) for r in res.results], axis=0
    ).reshape(N, C, H, W)
    if trace:
        return out, res
    return out
